# revision 17
# baseline (speedup 1.0000x reference)
"""Trainium2 Bass kernel for nn_MeshNetV0 (GNN message passing), 8 cores.

Strategy (pull-mode sharding):
  - Core c owns dst nodes [c*SHARD, (c+1)*SHARD) and all their in-edges.
  - Per conv layer: AllGather node features (bf16, rows padded to 256 B)
    into an HBM table [N_NODES, 128]; gpsimd dma_gather pulls per-edge
    source rows into edge-major SBUF tiles [128 slots, cols, 128]; per
    128-slot column a TensorE matmul  msg[:, :64].T @ S  (S = host-built
    weighted one-hot [128, w], bf16) computes the weighted segment-sum
    into feature-major PSUM [64, node-window]; the @W runs
    post-aggregation ((A h) W) and ACT applies bias+ReLU on the drain.
  - dma_gather indices are int16, so edges are grouped into NQ=4
    quarters by src range (each quarter <= 32768 table rows); quarter 0's
    columns tile each supertile exactly (start=True), quarters 1-3
    accumulate (start=False).
  - Combination 320->1024 + max-pool run locally; AllReduce-max merges;
    the BN-folded FC head is replicated on every core.

The column structure (node ranges per 128-slot column) is built from
max-over-cores degree sums so the instruction stream is identical on all
8 cores; only tensor contents (gather indices, S values) differ.
"""

import math
import numpy as np
import ml_dtypes

import concourse.bacc as bacc
import concourse.bass as bass
import concourse.mybir as mybir
import concourse.tile as tile
from concourse.bass_utils import run_bass_kernel_spmd
from concourse.library_config import mlp as mlp_lib

BF16 = ml_dtypes.bfloat16
AF = mybir.ActivationFunctionType
ALU = mybir.AluOpType

N_CORES = 8
N_NODES = 100000
NQ = 4               # src quarters (int16 index range)
NFEAT = 64
ROWPAD = 128         # table row width in bf16 elems (256 B)
NCLASS = 40
BN_EPS = 1e-5
SUPER = 512          # psum supertile width (nodes)
COMB_CHUNK = 2048    # nodes per combination-stage chunk


def _bf16(x):
    return np.ascontiguousarray(np.asarray(x).astype(BF16))


def _f32(x):
    return np.ascontiguousarray(np.asarray(x, dtype=np.float32))


# ----------------------------------------------------------------------
# Host preprocessing
# ----------------------------------------------------------------------

class Structure:
    pass


def build_structure(edge_src, edge_dst, edge_w, n_nodes, n_cores):
    """Partition edges by dst shard and src quarter, dst-sort, build the
    shared column structure plus per-core gather-index / S tensors.

    Slot order: [supertile s][quarter t][column k][slot]. Every column is
    padded to 128 slots. Returns a Structure with:
      - sup_bounds: [(s0, s1)]
      - plan: per (s, t): list of column node-spans [(a, b)], all within
        [s0, s1), disjoint, covering [s0, s1) exactly.
      - idx16[c]: int16 [128, TOTSLOT // 16] gather indices (wrapped, x8)
      - sval[c]: bf16 [128, SUMW] S values, column-major by (s, t, k)
      - per (s,t): slot offset, n columns, S offset
    """
    shard = n_nodes // n_cores
    qsz = n_nodes // NQ
    st = Structure()
    st.shard = shard
    st.qsz = qsz
    nsup = math.ceil(shard / SUPER)
    st.nsup = nsup
    st.sup_bounds = [(s * SUPER, min((s + 1) * SUPER, shard)) for s in range(nsup)]

    # per core, per quarter: dst-sorted edge lists + per-node degs
    deg = np.zeros((n_cores, NQ, shard), np.int64)
    edata = [[None] * NQ for _ in range(n_cores)]
    for c in range(n_cores):
        mc = (edge_dst // shard) == c
        srcc = edge_src[mc]
        dstc = edge_dst[mc] - c * shard
        wc = edge_w[mc]
        tq = srcc // qsz
        for t in range(NQ):
            m = tq == t
            src, dst, w = srcc[m], dstc[m], wc[m]
            o = np.argsort(dst, kind="stable")
            src, dst, w = src[o], dst[o], w[o]
            deg[c, t] = np.bincount(dst, minlength=shard)
            edata[c][t] = (src, dst, w)

    dcum = np.zeros((n_cores, NQ, shard + 1), np.int64)
    dcum[:, :, 1:] = np.cumsum(deg, axis=2)

    # greedy columns per (s, t): consecutive node runs with
    # max-over-cores edge count <= 128.
    st.plan = {}
    for s, (s0, s1) in enumerate(st.sup_bounds):
        for t in range(NQ):
            colspans = []
            a = s0
            while a < s1:
                b = a + 1
                while b < s1:
                    if (dcum[:, t, b + 1] - dcum[:, t, a]).max() > 128:
                        break
                    b += 1
                colspans.append((a, b))
                a = b
            st.plan[(s, t)] = colspans

    # layout: slots + S widths
    st.order = [(s, t) for s in range(nsup) for t in range(NQ)]
    st.col_off = {}   # (s,t) -> column index offset
    st.s_off = {}     # (s,t) -> S free-dim offset
    ncol = 0
    sumw = 0
    for (s, t) in st.order:
        st.col_off[(s, t)] = ncol
        st.s_off[(s, t)] = sumw
        ncol += len(st.plan[(s, t)])
        sumw += st.sup_bounds[s][1] - st.sup_bounds[s][0]
    st.ncol = ncol
    st.sumw = sumw
    totslot = ncol * 128
    st.totslot = totslot

    idx_all, sv_all = [], []
    for c in range(n_cores):
        idxf = np.zeros(totslot, np.int16)
        S = np.zeros((128, sumw), np.float32)
        for (s, t) in st.order:
            src, dst, w = edata[c][t]
            cum = dcum[c, t]
            co = st.col_off[(s, t)]
            so = st.s_off[(s, t)]
            s0 = st.sup_bounds[s][0]
            for k, (a, b) in enumerate(st.plan[(s, t)]):
                e0, e1 = int(cum[a]), int(cum[b])
                n = e1 - e0
                base = (co + k) * 128
                idxf[base : base + n] = (src[e0:e1] - t * qsz).astype(np.int16)
                S[np.arange(n), so + (a - s0) + (dst[e0:e1] - a)] = w[e0:e1]
        # wrap: idx i -> [i%16 (replicated to 8 groups), i//16]
        iw = idxf.reshape(-1, 16).T  # [16, totslot/16]
        idx16 = np.tile(iw, (8, 1))  # [128, totslot/16]
        idx_all.append(np.ascontiguousarray(idx16))
        sv_all.append(_bf16(S))
    st.fill = float(edge_src.size / (n_cores * totslot))
    return st, idx_all, sv_all


# ----------------------------------------------------------------------
# Bass program
# ----------------------------------------------------------------------

def build_program(st, n_nodes, n_cores, feats=(64, 64, 64, 128)):
    shard = st.shard
    nsup = st.nsup
    dt = mybir.dt

    nc = bacc.Bacc(
        "TRN2", target_bir_lowering=False, debug=False, num_devices=n_cores
    )
    rg = [list(range(n_cores))]

    # ---------------- I/O ----------------
    xsh_d = nc.dram_tensor("xsh", [shard, ROWPAD], dt.bfloat16, kind="ExternalInput")
    idx_d = nc.dram_tensor(
        "idx", [128, st.totslot // 16], dt.int16, kind="ExternalInput"
    )
    sv_d = nc.dram_tensor("sv", [128, st.sumw], dt.bfloat16, kind="ExternalInput")
    wl_d = [
        nc.dram_tensor(f"wl{l}", [NFEAT, feats[l]], dt.bfloat16, kind="ExternalInput")
        for l in range(4)
    ]
    bl_d = [
        nc.dram_tensor(f"bl{l}", [feats[l], 1], dt.float32, kind="ExternalInput")
        for l in range(4)
    ]
    wcb_d = nc.dram_tensor("wcb", [128, 3 * 1024], dt.bfloat16, kind="ExternalInput")
    bcb_d = nc.dram_tensor("bcb", [128, 8], dt.float32, kind="ExternalInput")
    fw1_d = nc.dram_tensor("fw1", [128, 8 * 512], dt.bfloat16, kind="ExternalInput")
    fb1_d = nc.dram_tensor("fb1", [128, 4], dt.float32, kind="ExternalInput")
    fw2_d = nc.dram_tensor("fw2", [128, 4 * 256], dt.bfloat16, kind="ExternalInput")
    fb2_d = nc.dram_tensor("fb2", [128, 2], dt.float32, kind="ExternalInput")
    fw3_d = nc.dram_tensor("fw3", [128, 2 * NCLASS], dt.bfloat16, kind="ExternalInput")
    fb3_d = nc.dram_tensor("fb3", [NCLASS, 1], dt.float32, kind="ExternalInput")
    identp_d = nc.dram_tensor(
        "identp", [NFEAT, ROWPAD], dt.bfloat16, kind="ExternalInput"
    )
    out_d = nc.dram_tensor("out", [1, NCLASS], dt.float32, kind="ExternalOutput")

    # ---------------- internal DRAM ----------------
    h_d = [nc.dram_tensor(f"h{l}", [shard, ROWPAD], dt.bfloat16) for l in range(4)]
    table_d = [
        nc.dram_tensor(f"table{l}", [n_nodes, ROWPAD], dt.bfloat16,
                       addr_space="Shared")
        for l in range(4)
    ]
    xiT_d = [
        nc.dram_tensor(f"xiT{l}", [feats[l], shard], dt.bfloat16) for l in range(4)
    ]
    cmax_d = nc.dram_tensor("cmax", [128, 8], dt.float32)
    gmax_d = nc.dram_tensor("gmax", [128, 8], dt.float32, addr_space="Shared")

    max_cols = max(len(st.plan[k]) for k in st.order)

    from contextlib import ExitStack

    with tile.TileContext(nc) as tc, ExitStack() as stk:
        cpool = stk.enter_context(tc.tile_pool(name="consts", bufs=1))
        xT_pool = stk.enter_context(tc.tile_pool(name="xT", bufs=1))
        small_pool = stk.enter_context(tc.tile_pool(name="small", bufs=1))
        conv_stk = ExitStack()
        msg_pool = conv_stk.enter_context(tc.tile_pool(name="msg", bufs=3))
        sv_pool = conv_stk.enter_context(tc.tile_pool(name="svp", bufs=3))
        ix_pool = conv_stk.enter_context(tc.tile_pool(name="ixp", bufs=3))
        agg_pool = conv_stk.enter_context(tc.tile_pool(name="aggsb", bufs=2))
        xnode_pool = conv_stk.enter_context(tc.tile_pool(name="xnode", bufs=2))
        ps_agg = conv_stk.enter_context(
            tc.tile_pool(name="ps_agg", bufs=2, space="PSUM"))
        ps_x = conv_stk.enter_context(
            tc.tile_pool(name="ps_x", bufs=2, space="PSUM"))
        ps_t = conv_stk.enter_context(
            tc.tile_pool(name="ps_t", bufs=2, space="PSUM"))

        nc.gpsimd.load_library(mlp_lib)

        # ---- load constants ----
        wl_sb, bl_sb = [], []
        for l in range(4):
            w = cpool.tile([NFEAT, feats[l]], dt.bfloat16, tag=f"wl{l}")
            nc.sync.dma_start(out=w[:], in_=wl_d[l][:, :])
            b = cpool.tile([feats[l], 1], dt.float32, tag=f"bl{l}")
            nc.sync.dma_start(out=b[:], in_=bl_d[l][:, :])
            wl_sb.append(w)
            bl_sb.append(b)
        wcb_sb = cpool.tile([128, 3 * 1024], dt.bfloat16, tag="wcb")
        nc.sync.dma_start(out=wcb_sb[:], in_=wcb_d[:, :])
        bcb_sb = cpool.tile([128, 8], dt.float32, tag="bcb")
        nc.sync.dma_start(out=bcb_sb[:], in_=bcb_d[:, :])
        fw1_sb = cpool.tile([128, 8 * 512], dt.bfloat16, tag="fw1")
        nc.sync.dma_start(out=fw1_sb[:], in_=fw1_d[:, :])
        fb1_sb = cpool.tile([128, 4], dt.float32, tag="fb1")
        nc.sync.dma_start(out=fb1_sb[:], in_=fb1_d[:, :])
        fw2_sb = cpool.tile([128, 4 * 256], dt.bfloat16, tag="fw2")
        nc.sync.dma_start(out=fw2_sb[:], in_=fw2_d[:, :])
        fb2_sb = cpool.tile([128, 2], dt.float32, tag="fb2")
        nc.sync.dma_start(out=fb2_sb[:], in_=fb2_d[:, :])
        fw3_sb = cpool.tile([128, 2 * NCLASS], dt.bfloat16, tag="fw3")
        nc.sync.dma_start(out=fw3_sb[:], in_=fw3_d[:, :])
        fb3_sb = cpool.tile([NCLASS, 1], dt.float32, tag="fb3")
        nc.sync.dma_start(out=fb3_sb[:], in_=fb3_d[:, :])
        identp_sb = cpool.tile([NFEAT, ROWPAD], dt.bfloat16, tag="identp")
        nc.sync.dma_start(out=identp_sb[:], in_=identp_d[:, :])

        # layer-0 AllGather input = x shard (already padded)
        nc.sync.dma_start(out=h_d[0][:, :], in_=xsh_d[:, :])

        ntile_tr = math.ceil(shard / 128)

        for l in range(4):
            nf = feats[l]
            nc.gpsimd.collective_compute(
                "AllGather",
                ALU.bypass,
                replica_groups=rg,
                ins=[h_d[l][:, :]],
                outs=[table_d[l][:, :]],
            )
            xT_full = xT_pool.tile([128, shard], dt.bfloat16, tag="xT")
            xT_sb = xT_full[:nf, :]
            for s in range(nsup):
                s0, s1 = st.sup_bounds[s]
                sw = s1 - s0
                pagg = ps_agg.tile([64, SUPER], dt.float32, tag="pagg")
                for t in range(NQ):
                    colspans = st.plan[(s, t)]
                    ncols = len(colspans)
                    nidx = ncols * 128
                    co = st.col_off[(s, t)]
                    so = st.s_off[(s, t)]
                    # stream idx + S for this (s, t)
                    ix = ix_pool.tile([128, max_cols * 8], dt.int16, tag="ix")
                    nc.sync.dma_start(
                        out=ix[:, : nidx // 16],
                        in_=idx_d[:, co * 8 : co * 8 + nidx // 16],
                    )
                    sv = sv_pool.tile([128, SUPER], dt.bfloat16, tag="sv")
                    nc.sync.dma_start(
                        out=sv[:, :sw], in_=sv_d[:, so : so + sw]
                    )
                    msg = msg_pool.tile(
                        [128, max_cols * ROWPAD], dt.bfloat16, tag="msg"
                    )
                    nc.gpsimd.dma_gather(
                        msg[:, : ncols * ROWPAD].rearrange(
                            "p (c e) -> p c e", e=ROWPAD
                        ),
                        table_d[l][t * st.qsz : (t + 1) * st.qsz, :],
                        ix[:, : nidx // 16],
                        nidx,
                        nidx,
                        ROWPAD,
                        single_packet=False,
                    )
                    for k, (a, b) in enumerate(colspans):
                        # start=True only on the very first matmul of this
                        # psum tile: it zeroes the whole bank; all later
                        # matmuls accumulate (onto zero for first writes).
                        nc.tensor.matmul(
                            out=pagg[:, a - s0 : b - s0],
                            lhsT=msg[:, k * ROWPAD : k * ROWPAD + NFEAT],
                            rhs=sv[:, a - s0 : b - s0],
                            start=(t == 0 and k == 0),
                            stop=(t == NQ - 1 and k == len(colspans) - 1),
                            skip_group_check=True,
                        )
                agg_sb = agg_pool.tile([64, SUPER], dt.bfloat16, tag="aggsb")
                nc.vector.tensor_copy(out=agg_sb[:, :sw], in_=pagg[:, :sw])
                pxt = ps_x.tile([128, SUPER], dt.float32, tag="px")
                px = pxt[:nf, :]
                nc.tensor.matmul(
                    out=px[:, :sw],
                    lhsT=wl_sb[l][:, :],
                    rhs=agg_sb[:, :sw],
                    start=True,
                    stop=True,
                )
                nc.scalar.activation(
                    out=xT_sb[:, s0:s1],
                    in_=px[:, :sw],
                    func=AF.Relu,
                    bias=bl_sb[l][:, :],
                    scale=1.0,
                )
            # spill xT for the combination stage
            nc.sync.dma_start(out=xiT_d[l][:, :], in_=xT_sb[:, :])
            if l < 3:
                # node-major (row-padded) transpose via TensorE into h_d[l+1]
                xnode = xnode_pool.tile(
                    [128, ntile_tr * ROWPAD], dt.bfloat16, tag="xnode"
                )
                for t in range(ntile_tr):
                    n0 = t * 128
                    n1 = min(n0 + 128, shard)
                    pt = ps_t.tile([128, ROWPAD], dt.float32, tag="pt")
                    nc.tensor.matmul(
                        out=pt[: n1 - n0, :],
                        lhsT=xT_sb[:, n0:n1],
                        rhs=identp_sb[:, :],
                        start=True,
                        stop=True,
                    )
                    nc.vector.tensor_copy(
                        out=xnode[: n1 - n0, t * ROWPAD : (t + 1) * ROWPAD],
                        in_=pt[: n1 - n0, :],
                    )
                nfull = shard // 128
                nc.sync.dma_start(
                    out=h_d[l + 1][: nfull * 128, :].rearrange(
                        "(t p) f -> p t f", p=128
                    ),
                    in_=xnode[:, : nfull * ROWPAD].rearrange(
                        "p (t f) -> p t f", f=ROWPAD
                    ),
                )
                if shard % 128:
                    rem = shard % 128
                    nc.sync.dma_start(
                        out=h_d[l + 1][nfull * 128 :, :],
                        in_=xnode[:rem, nfull * ROWPAD : (nfull + 1) * ROWPAD],
                    )

        # ---------------- combination + maxpool ----------------
        conv_stk.close()
        comb_pool = stk.enter_context(tc.tile_pool(name="comb", bufs=2))
        ps_c = stk.enter_context(tc.tile_pool(name="ps_c", bufs=3, space="PSUM"))
        ps_h = stk.enter_context(tc.tile_pool(name="ps_h", bufs=2, space="PSUM"))
        cmax_sb = small_pool.tile([128, 8], dt.float32, tag="cmax")
        nchunk = math.ceil(shard / COMB_CHUNK)
        for ci in range(nchunk):
            n0 = ci * COMB_CHUNK
            n1 = min(n0 + COMB_CHUNK, shard)
            cw = n1 - n0
            q0 = comb_pool.tile([128, COMB_CHUNK], dt.bfloat16, tag="q0")
            nc.sync.dma_start(out=q0[0:64, :cw], in_=xiT_d[0][:, n0:n1])
            nc.sync.dma_start(out=q0[64:128, :cw], in_=xiT_d[1][:, n0:n1])
            q1 = comb_pool.tile([128, COMB_CHUNK], dt.bfloat16, tag="q1")
            nc.sync.dma_start(out=q1[0:64, :cw], in_=xiT_d[2][:, n0:n1])
            nc.sync.dma_start(out=q1[64:128, :cw], in_=xiT_d[3][0:64, n0:n1])
            q2 = comb_pool.tile([64, COMB_CHUNK], dt.bfloat16, tag="q2")
            nc.sync.dma_start(out=q2[:, :cw], in_=xiT_d[3][64:128, n0:n1])
            for ns in range(math.ceil(cw / SUPER)):
                c0 = ns * SUPER
                c1 = min(c0 + SUPER, cw)
                for m in range(8):
                    pc = ps_c.tile([128, SUPER], dt.float32, tag="pc")
                    nc.tensor.matmul(
                        out=pc[:, : c1 - c0],
                        lhsT=wcb_sb[0:128, m * 128 : (m + 1) * 128],
                        rhs=q0[:, c0:c1],
                        start=True, stop=False,
                    )
                    nc.tensor.matmul(
                        out=pc[:, : c1 - c0],
                        lhsT=wcb_sb[0:128, 1024 + m * 128 : 1024 + (m + 1) * 128],
                        rhs=q1[:, c0:c1],
                        start=False, stop=False,
                    )
                    nc.tensor.matmul(
                        out=pc[:, : c1 - c0],
                        lhsT=wcb_sb[0:64, 2048 + m * 128 : 2048 + (m + 1) * 128],
                        rhs=q2[:, c0:c1],
                        start=False, stop=True,
                    )
                    red = small_pool.tile([128, 1], dt.float32, tag="red")
                    nc.vector.tensor_reduce(
                        out=red[:, :],
                        in_=pc[:, : c1 - c0],
                        axis=mybir.AxisListType.X,
                        op=ALU.max,
                    )
                    if ci == 0 and ns == 0:
                        nc.vector.tensor_copy(out=cmax_sb[:, m : m + 1], in_=red[:, :])
                    else:
                        nc.vector.tensor_tensor(
                            out=cmax_sb[:, m : m + 1],
                            in0=cmax_sb[:, m : m + 1],
                            in1=red[:, :],
                            op=ALU.max,
                        )

        # ---------------- all-reduce max + head ----------------
        nc.sync.dma_start(out=cmax_d[:, :], in_=cmax_sb[:, :])
        nc.gpsimd.collective_compute(
            "AllReduce", ALU.max, replica_groups=rg,
            ins=[cmax_d[:, :]], outs=[gmax_d[:, :]],
        )
        gmax_sb = small_pool.tile([128, 8], dt.float32, tag="gmax")
        nc.sync.dma_start(out=gmax_sb[:, :], in_=gmax_d[:, :])
        h0 = small_pool.tile([128, 8], dt.bfloat16, tag="h0")
        hpre = small_pool.tile([128, 8], dt.float32, tag="hpre")
        nc.vector.tensor_tensor(
            out=hpre[:, :], in0=gmax_sb[:, :], in1=bcb_sb[:, :], op=ALU.add
        )
        nc.vector.tensor_relu(out=h0[:, :], in_=hpre[:, :])

        h1 = small_pool.tile([128, 4], dt.bfloat16, tag="h1")
        for o in range(4):
            ph = ps_h.tile([128, 1], dt.float32, tag="ph")
            for q in range(8):
                nc.tensor.matmul(
                    out=ph[:, :],
                    lhsT=fw1_sb[:, q * 512 + o * 128 : q * 512 + (o + 1) * 128],
                    rhs=h0[:, q : q + 1],
                    start=(q == 0), stop=(q == 7),
                )
            nc.scalar.activation(
                out=h1[:, o : o + 1], in_=ph[:, :], func=AF.Relu,
                bias=fb1_sb[:, o : o + 1], scale=1.0,
            )
        h2 = small_pool.tile([128, 2], dt.bfloat16, tag="h2")
        for o in range(2):
            ph = ps_h.tile([128, 1], dt.float32, tag="ph")
            for q in range(4):
                nc.tensor.matmul(
                    out=ph[:, :],
                    lhsT=fw2_sb[:, q * 256 + o * 128 : q * 256 + (o + 1) * 128],
                    rhs=h1[:, q : q + 1],
                    start=(q == 0), stop=(q == 3),
                )
            nc.scalar.activation(
                out=h2[:, o : o + 1], in_=ph[:, :], func=AF.Relu,
                bias=fb2_sb[:, o : o + 1], scale=1.0,
            )
        po = ps_h.tile([NCLASS, 1], dt.float32, tag="po")
        for q in range(2):
            nc.tensor.matmul(
                out=po[:, :],
                lhsT=fw3_sb[:, q * NCLASS : (q + 1) * NCLASS],
                rhs=h2[:, q : q + 1],
                start=(q == 0), stop=(q == 1),
            )
        out_sb = small_pool.tile([NCLASS, 1], dt.float32, tag="outsb")
        nc.vector.tensor_tensor(
            out=out_sb[:, :], in0=po[:, :], in1=fb3_sb[:, :], op=ALU.add
        )
        nc.sync.dma_start(
            out=out_d.ap().rearrange("a b -> b a"), in_=out_sb[:, :]
        )

    nc.compile()
    return nc


# ----------------------------------------------------------------------
# Host wrapper
# ----------------------------------------------------------------------

def make_inputs(inputs, st, idx_all, sv_all, n_nodes, n_cores):
    shard = n_nodes // n_cores
    x = _f32(inputs["x"])
    s_bn = lambda g: _f32(g) / np.sqrt(np.float32(1.0) + np.float32(BN_EPS))

    def pack_lhsT(w, kdim, mdim):  # w: [out, in] -> [128, (kdim/128)*mdim]
        nq = kdim // 128
        arr = np.zeros((128, nq * mdim), np.float32)
        for q in range(nq):
            arr[:, q * mdim : (q + 1) * mdim] = w[:, q * 128 : (q + 1) * 128].T
        return _bf16(arr)

    s1, s2 = s_bn(inputs["g1"]), s_bn(inputs["g2"])
    w1 = _f32(inputs["fc1_w"]) * s1[:, None]
    b1 = _f32(inputs["fc1_b"]) * s1 + _f32(inputs["be1"])
    w2 = _f32(inputs["fc2_w"]) * s2[:, None]
    b2 = _f32(inputs["fc2_b"]) * s2 + _f32(inputs["be2"])
    w3 = _f32(inputs["fc3_w"])
    b3 = _f32(inputs["fc3_b"])

    fw1 = pack_lhsT(w1, 1024, 512)
    fb1 = np.zeros((128, 4), np.float32)
    for o in range(4):
        fb1[:, o] = b1[o * 128 : (o + 1) * 128]
    fw2 = pack_lhsT(w2, 512, 256)
    fb2 = np.zeros((128, 2), np.float32)
    for o in range(2):
        fb2[:, o] = b2[o * 128 : (o + 1) * 128]
    fw3 = pack_lhsT(w3, 256, NCLASS)
    fb3 = b3[:, None]

    wcb = _f32(inputs["Wcb"])  # [320, 1024]
    wcb_p = np.zeros((128, 3 * 1024), np.float32)
    wcb_p[0:128, 0:1024] = wcb[0:128]
    wcb_p[0:128, 1024:2048] = wcb[128:256]
    wcb_p[0:64, 2048:3072] = wcb[256:320]
    bcb = _f32(inputs["bcb"])
    bcb_p = np.zeros((128, 8), np.float32)
    for m in range(8):
        bcb_p[:, m] = bcb[m * 128 : (m + 1) * 128]

    identp = np.zeros((NFEAT, ROWPAD), np.float32)
    identp[:, :NFEAT] = np.eye(NFEAT)

    common = dict(
        wcb=_bf16(wcb_p),
        bcb=bcb_p,
        fw1=fw1, fb1=fb1, fw2=fw2, fb2=fb2, fw3=fw3, fb3=fb3,
        identp=_bf16(identp),
    )
    for l in range(4):
        common[f"wl{l}"] = _bf16(inputs[f"W{l + 1}"])
        common[f"bl{l}"] = _f32(inputs[f"b{l + 1}"])[:, None]

    in_maps = []
    for c in range(n_cores):
        m = dict(common)
        m["sv"] = sv_all[c]
        m["idx"] = idx_all[c]
        xp = np.zeros((shard, ROWPAD), np.float32)
        xp[:, :NFEAT] = x[c * shard : (c + 1) * shard]
        m["xsh"] = _bf16(xp)
        in_maps.append(m)
    return in_maps


LAST_EXEC_NS = None


def bench_time(nc, in_maps, n_cores, iters=6):
    """Median wall time per execution with device-resident inputs
    (replicates bass2jax.run_bass_via_pjrt's jit; NEFF already cached)."""
    import time
    import jax
    import concourse.mybir as mb
    from concourse import bass2jax
    from jax.sharding import Mesh, PartitionSpec, NamedSharding
    from jax.experimental.shard_map import shard_map

    bass2jax.install_neuronx_cc_hook()
    partition_name = (
        nc.partition_id_tensor.name if nc.partition_id_tensor else None
    )
    in_names, out_names, out_avals, zero_outs = [], [], [], []
    for alloc in nc.m.functions[0].allocations:
        if not isinstance(alloc, mb.MemoryLocationSet):
            continue
        name = alloc.memorylocations[0].name
        if alloc.kind == "ExternalInput":
            if name != partition_name:
                in_names.append(name)
        elif alloc.kind == "ExternalOutput":
            out_names.append(name)
            shape = tuple(alloc.tensor_shape)
            dtype = mb.dt.np(alloc.dtype)
            out_avals.append(jax.core.ShapedArray(shape, dtype))
            zero_outs.append(np.zeros(shape, dtype))
    n_params = len(in_names)
    donate = tuple(range(n_params, n_params + len(zero_outs)))

    def _body(*args):
        operands = list(args)
        if partition_name is not None:
            operands.append(bass2jax.partition_id_tensor())
        outs = bass2jax._bass_exec_p.bind(
            *operands,
            out_avals=tuple(out_avals),
            in_names=tuple(in_names + out_names
                           + ([partition_name] if partition_name else [])),
            out_names=tuple(out_names),
            lowering_input_output_aliases=(),
            sim_require_finite=True,
            sim_require_nnan=True,
            nc=nc,
        )
        return tuple(outs)

    devices = jax.devices()[:n_cores]
    mesh = Mesh(np.asarray(devices), ("core",))
    sharded = jax.jit(
        shard_map(
            _body, mesh=mesh,
            in_specs=(PartitionSpec("core"),) * (n_params + len(zero_outs)),
            out_specs=(PartitionSpec("core"),) * len(out_names),
            check_rep=False,
        ),
        donate_argnums=donate, keep_unused=True,
    )
    sh = NamedSharding(mesh, PartitionSpec("core"))
    dev_in = [
        jax.device_put(
            np.concatenate([np.asarray(in_maps[c][n]) for c in range(n_cores)], 0),
            sh,
        )
        for n in in_names
    ]
    times = []
    for it in range(iters):
        zo = [
            jax.device_put(
                np.zeros((n_cores * z.shape[0], *z.shape[1:]), z.dtype), sh
            )
            for z in zero_outs
        ]
        jax.block_until_ready(zo)
        t0 = time.perf_counter()
        out = sharded(*dev_in, *zo)
        jax.block_until_ready(out)
        times.append(time.perf_counter() - t0)
    times.sort()
    return times[len(times) // 2], times


def kernel(**inputs):
    global LAST_EXEC_NS
    import os
    import time

    edge_src = np.asarray(inputs["edge_src"])
    edge_dst = np.asarray(inputs["edge_dst"])
    edge_w = _f32(inputs["edge_w"])

    t0 = time.time()
    st, idx_all, sv_all = build_structure(
        edge_src, edge_dst, edge_w, N_NODES, N_CORES
    )
    t1 = time.time()
    nc = build_program(st, N_NODES, N_CORES)
    t2 = time.time()
    in_maps = make_inputs(inputs, st, idx_all, sv_all, N_NODES, N_CORES)
    trace = bool(int(os.environ.get("KERNEL_TRACE", "0")))
    res = run_bass_kernel_spmd(
        nc, in_maps, core_ids=list(range(N_CORES)), trace=trace
    )
    t3 = time.time()
    LAST_EXEC_NS = res.exec_time_ns
    print(
        f"[kernel] fill={st.fill:.3f} ncol={st.ncol} "
        f"prep={t1 - t0:.1f}s build+compile={t2 - t1:.1f}s "
        f"run={t3 - t2:.1f}s exec_ns={res.exec_time_ns}"
    )
    if int(os.environ.get("KERNEL_BENCH", "0")):
        med, times = bench_time(nc, in_maps, N_CORES)
        LAST_EXEC_NS = int(med * 1e9)
        print(f"[kernel] bench times: {[f'{t*1e3:.2f}ms' for t in times]}")
    return np.asarray(res.results[0]["out"], dtype=np.float32)


if __name__ == "__main__":
    data = dict(np.load("/root/problem/inputs.npz"))
    out = kernel(**data)
    print(out)
